# revision 10
# baseline (speedup 1.0000x reference)
"""Multi-head causal attention on 8 Trainium2 NeuronCores.

Problem: B=2, T=2048, C=1024, H=16, HS=64 (fp32), causal mask.

Sharding: 8 cores = 2 batches x 4 head-groups (4 heads each). Each core
computes q/k/v projections + attention + its partial output projection for
its 4 heads of its batch; the host sums the 4 per-batch partials (the
all-reduce of the tensor-parallel output projection) and adds the bias.

Per-core kernel dataflow (everything "transposed", T on the free axis):
  qT/kT [heads(64)x2, T] = W.T @ xT          (PE, K=C chunks of 128)
  v     [T, 64+ones]                         (PE)
  sT    [ts=128, tq=512] = kT.T-slice @ qT   (PE)  -> exp(s/8) (ACT)
  causal: multiplicative 0/1 mask tiles on the 4 diagonal ts-chunks (DVE)
  attnT_aug [65, tq] += v_aug.T @ expT       (PE, ones column => row 64 = softmax denom)
  recip = 1/denom (DVE), broadcast over 64 partitions via K=1 matmul (PE)
  attnT = attnT_aug[0:64] * recip            (DVE)  (odd head -> partition-shift DMA)
  y_partial [tq, C] = attnT_pair.T @ wproj   (PE, K=128 per head-pair)

float32r = full-precision fp32 matmul at 1 cycle/row (vs 4 for plain fp32).
"""

import numpy as np

B, T, C, H, HS = 2, 2048, 1024, 16, 64
NCORES = 8
HPC = 4            # heads per core
NKC = C // 128     # contraction chunks (8)
NJ = T // 512      # tq chunks (4)
NTS = T // 128     # ts chunks (16)

_NC_CACHE = {}


def _build_nc():
    if "nc" in _NC_CACHE:
        return _NC_CACHE["nc"]
    from contextlib import ExitStack
    import concourse.bass as bass
    from concourse import bacc, tile, mybir

    f32 = mybir.dt.float32
    f32r = mybir.dt.float32r
    EXP = mybir.ActivationFunctionType.Exp

    nc = bacc.Bacc("TRN2", target_bir_lowering=False, debug=False,
                   enable_asserts=False, num_devices=NCORES)

    xT_d = nc.dram_tensor("xT", (C, T), f32, kind="ExternalInput").ap()
    wq_d = nc.dram_tensor("wq_s", (C, HPC * HS), f32, kind="ExternalInput").ap()
    wk_d = nc.dram_tensor("wk_s", (C, HPC * HS), f32, kind="ExternalInput").ap()
    wv_d = nc.dram_tensor("wv_s", (C, HPC * HS), f32, kind="ExternalInput").ap()
    wp_d = nc.dram_tensor("wp_s", (HPC * HS, C), f32, kind="ExternalInput").ap()
    y_d = nc.dram_tensor("y", (T, C), f32, kind="ExternalOutput").ap()

    scale = float(1.0 / np.sqrt(HS))

    with tile.TileContext(nc) as tc, ExitStack() as ctx:
        persist = ctx.enter_context(tc.tile_pool(name="persist", bufs=1))
        work = ctx.enter_context(tc.tile_pool(name="work", bufs=3))
        small = ctx.enter_context(tc.tile_pool(name="small", bufs=2))
        outp = ctx.enter_context(tc.tile_pool(name="outp", bufs=3))
        psp = ctx.enter_context(tc.tile_pool(name="psp", bufs=2, space="PSUM"))
        psaux = ctx.enter_context(tc.tile_pool(name="psaux", bufs=2, space="PSUM"))
        psatt = ctx.enter_context(tc.tile_pool(name="psatt", bufs=2, space="PSUM"))

        # ---- persistent SBUF tensors (f32r = fast-fp32 PE path, ~1.6e-4) ----
        xt = [persist.tile([128, T], f32r, tag=f"xt{c}", name=f"xt{c}") for c in range(NKC)]
        wq_sb = persist.tile([128, NKC, 256], f32r, tag="wq")
        wk_sb = persist.tile([128, NKC, 256], f32r, tag="wk")
        wv_sb = persist.tile([128, NKC, 256], f32r, tag="wv")
        wp_sb = persist.tile([128, 2, C], f32r, tag="wp")
        qT = [persist.tile([128, T], f32r, tag=f"qT{p}", name=f"qT{p}") for p in range(2)]
        kT = [persist.tile([128, T], f32r, tag=f"kT{p}", name=f"kT{p}") for p in range(2)]
        vt = [persist.tile([128, NTS * 65], f32r, tag=f"vt{h}", name=f"vt{h}") for h in range(HPC)]
        attnT = [persist.tile([128, T], f32r, tag=f"attnT{p}", name=f"attnT{p}") for p in range(2)]

        # ---- loads: weights first (small queues), xT split across two queues ----
        nc.gpsimd.dma_start(out=wq_sb, in_=wq_d.rearrange("(c p) m -> p c m", p=128).bitcast(f32r))
        nc.gpsimd.dma_start(out=wk_sb, in_=wk_d.rearrange("(c p) m -> p c m", p=128).bitcast(f32r))
        nc.gpsimd.dma_start(out=wv_sb, in_=wv_d.rearrange("(c p) m -> p c m", p=128).bitcast(f32r))
        nc.gpsimd.dma_start(out=wp_sb, in_=wp_d.rearrange("(k p) n -> p k n", p=128).bitcast(f32r))
        for c in range(NKC):
            eng = nc.sync if c % 2 == 0 else nc.scalar
            eng.dma_start(out=xt[c], in_=xT_d[c * 128:(c + 1) * 128, :].bitcast(f32r))

        ones16 = persist.tile([128, NTS, 1], f32, tag="ones16")
        nc.vector.memset(ones16, 1.0)

        # ---------- emission helpers ----------
        def qk_chain(pair, dst, w_sb, J):
            ps = psaux.tile([128, 512], f32, tag="aux", name=f"qk_{pair}_{J}")
            for c in range(NKC):
                nc.tensor.matmul(
                    ps,
                    lhsT=w_sb[:, c, 128 * pair:128 * pair + 128],
                    rhs=xt[c][:, 512 * J:512 * J + 512],
                    start=(c == 0), stop=(c == NKC - 1))
            nc.vector.tensor_copy(out=dst[:, 512 * J:512 * J + 512], in_=ps)

        def v_chain(t):
            ps = psaux.tile([128, 512], f32, tag="aux", name=f"v_{t}")
            for c in range(NKC):
                nc.tensor.matmul(
                    ps[:, 0:256],
                    lhsT=xt[c][:, 128 * t:128 * t + 128],
                    rhs=wv_sb[:, c, :],
                    start=(c == 0), stop=(c == NKC - 1))
            for h in range(HPC):
                nc.vector.tensor_copy(
                    out=vt[h][:, 65 * t:65 * t + 64], in_=ps[:, 64 * h:64 * h + 64])

        def att_block(pair, hh, J):
            h = 2 * pair + hh
            nch = 4 * J + 4
            pa = psatt.tile([65, 512], f32, tag="att", name=f"pa_{h}_{J}")
            for u in range(nch // 2):
                t0, t1 = 2 * u, 2 * u + 1
                ss = psp.tile([128, 1024], f32, tag="s", name=f"ss_{h}_{J}_{u}")
                for half, t in ((0, t0), (1, t1)):
                    nc.tensor.matmul(
                        ss[:, 512 * half:512 * half + 512],
                        lhsT=kT[pair][64 * hh:64 * hh + 64, 128 * t:128 * t + 128],
                        rhs=qT[pair][64 * hh:64 * hh + 64, 512 * J:512 * J + 512],
                        start=True, stop=True)
                et = work.tile([128, 1024], f32r, tag="et")
                nc.scalar.activation(out=et, in_=ss, func=EXP, scale=scale)
                for half, t in ((0, t0), (1, t1)):
                    if t >= 4 * J:
                        d = t - 4 * J
                        sl = et[:, 512 * half:512 * half + 512]
                        # keep el iff f >= p + 128*d:  (-1)*p + 1*f + (-128*d) >= 0
                        nc.gpsimd.affine_select(
                            out=sl, in_=sl,
                            compare_op=mybir.AluOpType.is_ge,
                            fill=0.0, base=-128 * d,
                            pattern=[[1, 512]], channel_multiplier=-1)
                for half, t in ((0, t0), (1, t1)):
                    nc.tensor.matmul(
                        pa,
                        lhsT=vt[h][:, 65 * t:65 * t + 65],
                        rhs=et[:, 512 * half:512 * half + 512],
                        start=(t == 0), stop=(t == nch - 1))
            sums = small.tile([1, 512], f32, tag="sums")
            nc.scalar.copy(sums, pa[64:65, :])
            bsums = small.tile([64, 512], f32, tag="bsums")
            nc.gpsimd.partition_broadcast(bsums, sums)
            recip = small.tile([64, 512], f32, tag="recip")
            nc.vector.reciprocal(recip, bsums)
            if hh == 0:
                nc.vector.tensor_mul(
                    attnT[pair][0:64, 512 * J:512 * J + 512], pa[0:64, :], recip)
            else:
                tmp = small.tile([64, 512], f32r, tag="tmp")
                nc.vector.tensor_mul(tmp, pa[0:64, :], recip)
                nc.sync.dma_start(
                    out=attnT[pair][64:128, 512 * J:512 * J + 512], in_=tmp)

        def proj_tile(m, n):
            py_ = psaux.tile([128, 512], f32, tag="aux", name=f"y_{m}_{n}")
            for pair in range(2):
                nc.tensor.matmul(
                    py_,
                    lhsT=attnT[pair][:, 128 * m:128 * m + 128],
                    rhs=wp_sb[:, pair, 512 * n:512 * n + 512],
                    start=(pair == 0), stop=(pair == 1))
            yo = outp.tile([128, 512], f32, tag="yo")
            nc.vector.tensor_copy(out=yo, in_=py_)
            nc.sync.dma_start(
                out=y_d[128 * m:128 * m + 128, 512 * n:512 * n + 512], in_=yo)

        # ---------- phase A: qk(pair0) + v(all 4 heads) ----------
        for J in range(NJ):
            qk_chain(0, qT[0], wq_sb, J)
        for t in range(0, NTS, 2):
            v_chain(t)
            v_chain(t + 1)
        for J in range(NJ):
            qk_chain(0, kT[0], wk_sb, J)
        for h in range(HPC):
            nc.vector.tensor_copy(
                out=vt[h].rearrange("p (t x) -> p t x", x=65)[:, :, 64:65],
                in_=ones16)

        # ---------- phase B: attention(pair0) interleaved with qk(pair1) ----------
        qk1_units = [(qT[1], wq_sb, J) for J in range(NJ)] + \
                    [(kT[1], wk_sb, J) for J in range(NJ)]
        ui = 0
        for J in range(NJ):
            for hh in range(2):
                att_block(0, hh, J)
                if ui < len(qk1_units):
                    dst, w_sb, Jx = qk1_units[ui]
                    qk_chain(1, dst, w_sb, Jx)
                    ui += 1
        while ui < len(qk1_units):
            dst, w_sb, Jx = qk1_units[ui]
            qk_chain(1, dst, w_sb, Jx)
            ui += 1

        # ---------- phase C: attention(pair1) interleaved with proj ----------
        for J in range(NJ):
            att_block(1, 0, J)
            att_block(1, 1, J)
            for m in range(4 * J, 4 * J + 4):
                for n in range(2):
                    proj_tile(m, n)

    nc.compile()
    _NC_CACHE["nc"] = nc
    return nc


def _make_mask01():
    m = np.zeros((4, 128, 512), dtype=np.float32)
    p = np.arange(128)[:, None]
    f = np.arange(512)[None, :]
    for d in range(4):
        m[d] = (f >= 128 * d + p).astype(np.float32)
    return m


def make_in_maps(x, wq, wk, wv, wproj):
    xTs = [np.ascontiguousarray(x[b].T) for b in range(B)]
    in_maps = []
    for core in range(NCORES):
        b, g = divmod(core, 4)
        hs = slice(4 * g, 4 * g + 4)
        in_maps.append({
            "xT": xTs[b],
            "wq_s": np.ascontiguousarray(wq[hs].transpose(1, 0, 2).reshape(C, HPC * HS)),
            "wk_s": np.ascontiguousarray(wk[hs].transpose(1, 0, 2).reshape(C, HPC * HS)),
            "wv_s": np.ascontiguousarray(wv[hs].transpose(1, 0, 2).reshape(C, HPC * HS)),
            "wp_s": np.ascontiguousarray(wproj[4 * g * HS:(4 * g + 4) * HS, :]),
        })
    return in_maps


def _assemble(results, bproj):
    y = np.zeros((B, T, C), dtype=np.float32)
    for core in range(NCORES):
        y[core // 4] += results[core]["y"]
    y += bproj.astype(np.float32)[None, None, :]
    return y


def _is_causal(attention_mask):
    tril = np.tril(np.ones((T, T), dtype=bool))
    return all(np.array_equal(attention_mask[b], tril) for b in range(B))


def _numpy_fallback(x, attention_mask, wq, wk, wv, wproj, bproj):
    x64 = x.astype(np.float32)
    q = np.einsum('btc,hcd->bhtd', x64, wq)
    k = np.einsum('btc,hcd->bhtd', x64, wk)
    v = np.einsum('btc,hcd->bhtd', x64, wv)
    wei = np.einsum('bhtd,bhsd->bhts', q, k) / np.sqrt(np.float32(HS))
    wei = np.where(attention_mask[:, None, :, :], wei, -np.inf)
    wei = wei - wei.max(axis=-1, keepdims=True)
    wei = np.exp(wei)
    wei = wei / wei.sum(axis=-1, keepdims=True)
    out = np.einsum('bhts,bhsd->bhtd', wei, v)
    out = out.transpose(0, 2, 1, 3).reshape(B, T, H * HS)
    return (out @ wproj + bproj).astype(np.float32)


def _install_ntff_hook():
    """Recreate the antenv.axon_hooks shim so trace=True works under axon."""
    import sys, types
    try:
        from antenv.axon_hooks import get_axon_ntff_profile_hook  # noqa
        return
    except ImportError:
        pass
    import antenv
    mod = types.ModuleType("antenv.axon_hooks")
    holder = [None]
    mod.set_axon_ntff_profile_hook = lambda h: holder.__setitem__(0, h)
    mod.get_axon_ntff_profile_hook = lambda: holder[0]
    sys.modules["antenv.axon_hooks"] = mod
    antenv.axon_hooks = mod
    if "/root/.axon_site" not in sys.path:
        sys.path.insert(0, "/root/.axon_site")
    from trn_agent_boot.trn_boot import _ntff_profile_via_ctypes
    mod.set_axon_ntff_profile_hook(_ntff_profile_via_ctypes("/opt/axon/libaxon_pjrt.so"))


def kernel(x, attention_mask, wq, wk, wv, wproj, bproj, _trace=False):
    x = np.asarray(x); attention_mask = np.asarray(attention_mask)
    wq = np.asarray(wq); wk = np.asarray(wk); wv = np.asarray(wv)
    wproj = np.asarray(wproj); bproj = np.asarray(bproj)

    if not _is_causal(attention_mask):
        return _numpy_fallback(x, attention_mask, wq, wk, wv, wproj, bproj)

    from concourse import bass_utils
    if _trace:
        _install_ntff_hook()
        bass_utils.upload_artifacts = lambda d: d
    nc = _build_nc()
    in_maps = make_in_maps(x, wq, wk, wv, wproj)
    res = bass_utils.run_bass_kernel_spmd(
        nc, in_maps, core_ids=list(range(NCORES)), trace=_trace)
    out = _assemble(res.results, bproj)
    if _trace:
        return out, res
    return out


# revision 12
# speedup vs baseline: 1.1407x; 1.1407x over previous
"""Multi-head causal attention on 8 Trainium2 NeuronCores.

Problem: B=2, T=2048, C=1024, H=16, HS=64 (fp32), causal mask.

Sharding: 8 cores = 2 batches x 4 head-groups (4 heads each). Each core
computes q/k/v projections + attention + its partial output projection for
its 4 heads of its batch; the host sums the 4 per-batch partials (the
all-reduce of the tensor-parallel output projection) and adds the bias.

Per-core kernel dataflow (everything "transposed", T on the free axis):
  qT/kT [heads(64)x2, T] = W.T @ xT          (PE, K=C chunks of 128)
  v     [T, 64+ones]                         (PE)
  sT    [ts=128, tq=512] = kT.T-slice @ qT   (PE)  -> exp(s/8) (ACT)
  causal: multiplicative 0/1 mask tiles on the 4 diagonal ts-chunks (DVE)
  attnT_aug [65, tq] += v_aug.T @ expT       (PE, ones column => row 64 = softmax denom)
  recip = 1/denom (DVE), broadcast over 64 partitions via K=1 matmul (PE)
  attnT = attnT_aug[0:64] * recip            (DVE)  (odd head -> partition-shift DMA)
  y_partial [tq, C] = attnT_pair.T @ wproj   (PE, K=128 per head-pair)

float32r = full-precision fp32 matmul at 1 cycle/row (vs 4 for plain fp32).
"""

import numpy as np

B, T, C, H, HS = 2, 2048, 1024, 16, 64
NCORES = 8
HPC = 4            # heads per core
NKC = C // 128     # contraction chunks (8)
NJ = T // 512      # tq chunks (4)
NTS = T // 128     # ts chunks (16)

_NC_CACHE = {}


def _build_nc():
    if "nc" in _NC_CACHE:
        return _NC_CACHE["nc"]
    from contextlib import ExitStack
    import concourse.bass as bass
    from concourse import bacc, tile, mybir

    f32 = mybir.dt.float32
    f32r = mybir.dt.float32r
    EXP = mybir.ActivationFunctionType.Exp

    nc = bacc.Bacc("TRN2", target_bir_lowering=False, debug=False,
                   enable_asserts=False, num_devices=NCORES)

    xT_d = nc.dram_tensor("xT", (C, T), f32, kind="ExternalInput").ap()
    wq_d = nc.dram_tensor("wq_s", (C, HPC * HS), f32, kind="ExternalInput").ap()
    wk_d = nc.dram_tensor("wk_s", (C, HPC * HS), f32, kind="ExternalInput").ap()
    wv_d = nc.dram_tensor("wv_s", (C, HPC * HS), f32, kind="ExternalInput").ap()
    wp_d = nc.dram_tensor("wp_s", (HPC * HS, C), f32, kind="ExternalInput").ap()
    y_d = nc.dram_tensor("y", (T, C), f32, kind="ExternalOutput").ap()

    scale = float(1.0 / np.sqrt(HS))

    with tile.TileContext(nc) as tc, ExitStack() as ctx:
        persist = ctx.enter_context(tc.tile_pool(name="persist", bufs=1))
        work = ctx.enter_context(tc.tile_pool(name="work", bufs=3))
        small = ctx.enter_context(tc.tile_pool(name="small", bufs=2))
        outp = ctx.enter_context(tc.tile_pool(name="outp", bufs=3))
        psp = ctx.enter_context(tc.tile_pool(name="psp", bufs=2, space="PSUM"))
        psaux = ctx.enter_context(tc.tile_pool(name="psaux", bufs=2, space="PSUM"))
        psatt = ctx.enter_context(tc.tile_pool(name="psatt", bufs=2, space="PSUM"))

        # ---- persistent SBUF tensors (f32r = fast-fp32 PE path, ~1.6e-4) ----
        xt = [persist.tile([128, T], f32r, tag=f"xt{c}", name=f"xt{c}") for c in range(NKC)]
        wq_sb = persist.tile([128, NKC, 256], f32r, tag="wq")
        wk_sb = persist.tile([128, NKC, 256], f32r, tag="wk")
        wv_sb = persist.tile([128, NKC, 256], f32r, tag="wv")
        wp_sb = persist.tile([128, 2, C], f32r, tag="wp")
        qT = [persist.tile([128, T], f32r, tag=f"qT{p}", name=f"qT{p}") for p in range(2)]
        kT = [persist.tile([128, T], f32r, tag=f"kT{p}", name=f"kT{p}") for p in range(2)]
        vt = [persist.tile([128, NTS * 65], f32r, tag=f"vt{h}", name=f"vt{h}") for h in range(HPC)]
        attnT = [persist.tile([128, T], f32r, tag=f"attnT{p}", name=f"attnT{p}") for p in range(2)]

        # ---- loads: chunked, interleaved in consumption order, 2 HW queues ----
        nc.gpsimd.dma_start(out=wp_sb, in_=wp_d.rearrange("(k p) n -> p k n", p=128).bitcast(f32r))
        for eng, par in ((nc.sync, 0), (nc.scalar, 1)):
            for c in range(par, NKC, 2):
                eng.dma_start(out=wq_sb[:, c, :],
                              in_=wq_d[c * 128:(c + 1) * 128, :].bitcast(f32r))
            for c in range(par, NKC, 2):
                eng.dma_start(out=xt[c], in_=xT_d[c * 128:(c + 1) * 128, :].bitcast(f32r))
            for c in range(par, NKC, 2):
                eng.dma_start(out=wv_sb[:, c, :],
                              in_=wv_d[c * 128:(c + 1) * 128, :].bitcast(f32r))
            for c in range(par, NKC, 2):
                eng.dma_start(out=wk_sb[:, c, :],
                              in_=wk_d[c * 128:(c + 1) * 128, :].bitcast(f32r))

        ones16 = persist.tile([128, NTS, 1], f32, tag="ones16")
        nc.vector.memset(ones16, 1.0)

        # ---------- emission helpers ----------
        filler = []     # queue of closures emitting independent PE work

        def pull(n):
            for _ in range(n):
                if filler:
                    filler.pop(0)()

        def qk_chain_units(pair, dst, w_sb, J, name):
            # split one 8-matmul accumulation chain into 4 filler units
            ps = psaux.tile([128, 512], f32, tag="aux", name=name)

            def unit(c0):
                def f():
                    for c in (c0, c0 + 1):
                        nc.tensor.matmul(
                            ps,
                            lhsT=w_sb[:, c, 128 * pair:128 * pair + 128],
                            rhs=xt[c][:, 512 * J:512 * J + 512],
                            start=(c == 0), stop=(c == NKC - 1))
                    if c0 == NKC - 2:
                        nc.vector.tensor_copy(
                            out=dst[:, 512 * J:512 * J + 512], in_=ps)
                return f
            return [unit(c0) for c0 in range(0, NKC, 2)]

        def qk_chain(pair, dst, w_sb, J, name):
            for u in qk_chain_units(pair, dst, w_sb, J, name):
                u()

        def v_chain(t):
            ps = psaux.tile([128, 512], f32, tag="aux", name=f"v_{t}")
            for c in range(NKC):
                nc.tensor.matmul(
                    ps[:, 0:256],
                    lhsT=xt[c][:, 128 * t:128 * t + 128],
                    rhs=wv_sb[:, c, :],
                    start=(c == 0), stop=(c == NKC - 1))
            for h in range(HPC):
                nc.vector.tensor_copy(
                    out=vt[h][:, 65 * t:65 * t + 64], in_=ps[:, 64 * h:64 * h + 64])

        def proj_tile(m, n):
            py_ = psaux.tile([128, 512], f32, tag="aux", name=f"y_{m}_{n}")
            for pair in range(2):
                nc.tensor.matmul(
                    py_,
                    lhsT=attnT[pair][:, 128 * m:128 * m + 128],
                    rhs=wp_sb[:, pair, 512 * n:512 * n + 512],
                    start=(pair == 0), stop=(pair == 1))
            yo = outp.tile([128, 512], f32, tag="yo")
            nc.vector.tensor_copy(out=yo, in_=py_)
            nc.sync.dma_start(
                out=y_d[128 * m:128 * m + 128, 512 * n:512 * n + 512], in_=yo)

        def att_block(pair, hh, J):
            h = 2 * pair + hh
            nch = 4 * J + 4
            pa = psatt.tile([65, 512], f32, tag="att", name=f"pa_{h}_{J}")
            pend = None          # (et, t0, t1) AV one step behind scores
            for u in range(nch // 2):
                t0, t1 = 2 * u, 2 * u + 1
                ss = psp.tile([128, 1024], f32, tag="s", name=f"ss_{h}_{J}_{u}")
                for half, t in ((0, t0), (1, t1)):
                    nc.tensor.matmul(
                        ss[:, 512 * half:512 * half + 512],
                        lhsT=kT[pair][64 * hh:64 * hh + 64, 128 * t:128 * t + 128],
                        rhs=qT[pair][64 * hh:64 * hh + 64, 512 * J:512 * J + 512],
                        start=True, stop=True)
                et = work.tile([128, 1024], f32r, tag="et", bufs=4)
                nc.scalar.activation(out=et, in_=ss, func=EXP, scale=scale)
                for half, t in ((0, t0), (1, t1)):
                    if t >= 4 * J:
                        d = t - 4 * J
                        sl = et[:, 512 * half:512 * half + 512]
                        # keep el iff f >= p + 128*d:  (-1)*p + 1*f - 128*d >= 0
                        nc.gpsimd.affine_select(
                            out=sl, in_=sl,
                            compare_op=mybir.AluOpType.is_ge,
                            fill=0.0, base=-128 * d,
                            pattern=[[1, 512]], channel_multiplier=-1)
                if pend is not None:
                    for half, t in ((0, pend[1]), (1, pend[2])):
                        nc.tensor.matmul(
                            pa, lhsT=vt[h][:, 65 * t:65 * t + 65],
                            rhs=pend[0][:, 512 * half:512 * half + 512],
                            start=(t == 0), stop=False)
                pend = (et, t0, t1)
                pull(1)
            for half, t in ((0, pend[1]), (1, pend[2])):
                nc.tensor.matmul(
                    pa, lhsT=vt[h][:, 65 * t:65 * t + 65],
                    rhs=pend[0][:, 512 * half:512 * half + 512],
                    start=(t == 0), stop=(t == nch - 1))
            sums = small.tile([1, 512], f32, tag="sums")
            nc.scalar.copy(sums, pa[64:65, :])
            bsums = small.tile([64, 512], f32, tag="bsums")
            nc.gpsimd.partition_broadcast(bsums, sums)
            recip = small.tile([64, 512], f32, tag="recip")
            nc.vector.reciprocal_approx_fast(out=recip, in_=bsums)
            if hh == 0:
                nc.vector.tensor_mul(
                    attnT[pair][0:64, 512 * J:512 * J + 512], pa[0:64, :], recip)
            else:
                tmp = small.tile([64, 512], f32r, tag="tmp")
                nc.vector.tensor_mul(tmp, pa[0:64, :], recip)
                nc.sync.dma_start(
                    out=attnT[pair][64:128, 512 * J:512 * J + 512], in_=tmp)

        # ---------- phase A: qk(pair0) + v(all 4 heads) ----------
        for J in range(NJ):
            qk_chain(0, qT[0], wq_sb, J, f"q0_{J}")
        for t in range(NTS):
            v_chain(t)
        for J in range(NJ):
            qk_chain(0, kT[0], wk_sb, J, f"k0_{J}")
        for h in range(HPC):
            nc.vector.tensor_copy(
                out=vt[h].rearrange("p (t x) -> p t x", x=65)[:, :, 64:65],
                in_=ones16)

        # ---------- phase B: attention(pair0), gaps filled with qk(pair1) ----------
        for J in range(NJ):
            filler.extend(qk_chain_units(1, qT[1], wq_sb, J, f"q1_{J}"))
        for J in range(NJ):
            filler.extend(qk_chain_units(1, kT[1], wk_sb, J, f"k1_{J}"))
        for J in range(NJ):
            for hh in range(2):
                att_block(0, hh, J)
        pull(len(filler))

        # ---------- phase C: attention(pair1), gaps filled with proj ----------
        for J in range(NJ):
            att_block(1, 0, J)
            att_block(1, 1, J)
            filler.extend(
                (lambda m=m, n=n: (lambda: proj_tile(m, n)))()
                for m in range(4 * J, 4 * J + 4) for n in range(2))
        pull(len(filler))

    nc.compile()
    _NC_CACHE["nc"] = nc
    return nc


def _make_mask01():
    m = np.zeros((4, 128, 512), dtype=np.float32)
    p = np.arange(128)[:, None]
    f = np.arange(512)[None, :]
    for d in range(4):
        m[d] = (f >= 128 * d + p).astype(np.float32)
    return m


def make_in_maps(x, wq, wk, wv, wproj):
    xTs = [np.ascontiguousarray(x[b].T) for b in range(B)]
    in_maps = []
    for core in range(NCORES):
        b, g = divmod(core, 4)
        hs = slice(4 * g, 4 * g + 4)
        in_maps.append({
            "xT": xTs[b],
            "wq_s": np.ascontiguousarray(wq[hs].transpose(1, 0, 2).reshape(C, HPC * HS)),
            "wk_s": np.ascontiguousarray(wk[hs].transpose(1, 0, 2).reshape(C, HPC * HS)),
            "wv_s": np.ascontiguousarray(wv[hs].transpose(1, 0, 2).reshape(C, HPC * HS)),
            "wp_s": np.ascontiguousarray(wproj[4 * g * HS:(4 * g + 4) * HS, :]),
        })
    return in_maps


def _assemble(results, bproj):
    y = np.zeros((B, T, C), dtype=np.float32)
    for core in range(NCORES):
        y[core // 4] += results[core]["y"]
    y += bproj.astype(np.float32)[None, None, :]
    return y


def _is_causal(attention_mask):
    tril = np.tril(np.ones((T, T), dtype=bool))
    return all(np.array_equal(attention_mask[b], tril) for b in range(B))


def _numpy_fallback(x, attention_mask, wq, wk, wv, wproj, bproj):
    x64 = x.astype(np.float32)
    q = np.einsum('btc,hcd->bhtd', x64, wq)
    k = np.einsum('btc,hcd->bhtd', x64, wk)
    v = np.einsum('btc,hcd->bhtd', x64, wv)
    wei = np.einsum('bhtd,bhsd->bhts', q, k) / np.sqrt(np.float32(HS))
    wei = np.where(attention_mask[:, None, :, :], wei, -np.inf)
    wei = wei - wei.max(axis=-1, keepdims=True)
    wei = np.exp(wei)
    wei = wei / wei.sum(axis=-1, keepdims=True)
    out = np.einsum('bhts,bhsd->bhtd', wei, v)
    out = out.transpose(0, 2, 1, 3).reshape(B, T, H * HS)
    return (out @ wproj + bproj).astype(np.float32)


def _install_ntff_hook():
    """Recreate the antenv.axon_hooks shim so trace=True works under axon."""
    import sys, types
    try:
        from antenv.axon_hooks import get_axon_ntff_profile_hook  # noqa
        return
    except ImportError:
        pass
    import antenv
    mod = types.ModuleType("antenv.axon_hooks")
    holder = [None]
    mod.set_axon_ntff_profile_hook = lambda h: holder.__setitem__(0, h)
    mod.get_axon_ntff_profile_hook = lambda: holder[0]
    sys.modules["antenv.axon_hooks"] = mod
    antenv.axon_hooks = mod
    if "/root/.axon_site" not in sys.path:
        sys.path.insert(0, "/root/.axon_site")
    from trn_agent_boot.trn_boot import _ntff_profile_via_ctypes
    mod.set_axon_ntff_profile_hook(_ntff_profile_via_ctypes("/opt/axon/libaxon_pjrt.so"))


def kernel(x, attention_mask, wq, wk, wv, wproj, bproj, _trace=False):
    x = np.asarray(x); attention_mask = np.asarray(attention_mask)
    wq = np.asarray(wq); wk = np.asarray(wk); wv = np.asarray(wv)
    wproj = np.asarray(wproj); bproj = np.asarray(bproj)

    if not _is_causal(attention_mask):
        return _numpy_fallback(x, attention_mask, wq, wk, wv, wproj, bproj)

    from concourse import bass_utils
    if _trace:
        _install_ntff_hook()
        bass_utils.upload_artifacts = lambda d: d
    nc = _build_nc()
    in_maps = make_in_maps(x, wq, wk, wv, wproj)
    res = bass_utils.run_bass_kernel_spmd(
        nc, in_maps, core_ids=list(range(NCORES)), trace=_trace)
    out = _assemble(res.results, bproj)
    if _trace:
        return out, res
    return out


# revision 13
# speedup vs baseline: 1.4043x; 1.2311x over previous
"""Multi-head causal attention on 8 Trainium2 NeuronCores.

Problem: B=2, T=2048, C=1024, H=16, HS=64 (fp32), causal mask.

Sharding: 8 cores = 2 batches x 4 head-groups (4 heads each). Each core
computes q/k/v projections + attention + its partial output projection for
its 4 heads of its batch; the host sums the 4 per-batch partials (the
all-reduce of the tensor-parallel output projection) and adds the bias.

Per-core kernel dataflow (everything "transposed", T on the free axis):
  qT/kT [heads(64)x2, T] = W.T @ xT          (PE, K=C chunks of 128)
  v     [T, 64+ones]                         (PE)
  sT    [ts=128, tq=512] = kT.T-slice @ qT   (PE)  -> exp(s/8) (ACT)
  causal: multiplicative 0/1 mask tiles on the 4 diagonal ts-chunks (DVE)
  attnT_aug [65, tq] += v_aug.T @ expT       (PE, ones column => row 64 = softmax denom)
  recip = 1/denom (DVE), broadcast over 64 partitions via K=1 matmul (PE)
  attnT = attnT_aug[0:64] * recip            (DVE)  (odd head -> partition-shift DMA)
  y_partial [tq, C] = attnT_pair.T @ wproj   (PE, K=128 per head-pair)

float32r = full-precision fp32 matmul at 1 cycle/row (vs 4 for plain fp32).
"""

import numpy as np

B, T, C, H, HS = 2, 2048, 1024, 16, 64
NCORES = 8
HPC = 4            # heads per core
NKC = C // 128     # contraction chunks (8)
NJ = T // 512      # tq chunks (4)
NTS = T // 128     # ts chunks (16)

_NC_CACHE = {}


def _build_nc():
    if "nc" in _NC_CACHE:
        return _NC_CACHE["nc"]
    from contextlib import ExitStack
    import concourse.bass as bass
    from concourse import bacc, tile, mybir

    f32 = mybir.dt.float32
    f32r = mybir.dt.float32r
    EXP = mybir.ActivationFunctionType.Exp

    nc = bacc.Bacc("TRN2", target_bir_lowering=False, debug=False,
                   enable_asserts=False, num_devices=NCORES)

    xT_d = nc.dram_tensor("xT", (C, T), f32, kind="ExternalInput").ap()
    wq_d = nc.dram_tensor("wq_s", (C, HPC * HS), f32, kind="ExternalInput").ap()
    wk_d = nc.dram_tensor("wk_s", (C, HPC * HS), f32, kind="ExternalInput").ap()
    wv_d = nc.dram_tensor("wv_s", (C, HPC * HS), f32, kind="ExternalInput").ap()
    wp_d = nc.dram_tensor("wp_s", (HPC * HS, C), f32, kind="ExternalInput").ap()
    y_d = nc.dram_tensor("y", (T, C), f32, kind="ExternalOutput").ap()

    scale = float(1.0 / np.sqrt(HS))

    with tile.TileContext(nc) as tc, ExitStack() as ctx:
        persist = ctx.enter_context(tc.tile_pool(name="persist", bufs=1))
        work = ctx.enter_context(tc.tile_pool(name="work", bufs=3))
        small = ctx.enter_context(tc.tile_pool(name="small", bufs=2))
        outp = ctx.enter_context(tc.tile_pool(name="outp", bufs=2))
        psp = ctx.enter_context(tc.tile_pool(name="psp", bufs=2, space="PSUM"))
        psaux = ctx.enter_context(tc.tile_pool(name="psaux", bufs=2, space="PSUM"))
        psatt = ctx.enter_context(tc.tile_pool(name="psatt", bufs=2, space="PSUM"))

        # ---- persistent SBUF tensors (f32r = fast-fp32 PE path, ~1.6e-4) ----
        xt = [persist.tile([128, T], f32r, tag=f"xt{c}", name=f"xt{c}") for c in range(NKC)]
        wq_sb = persist.tile([128, NKC, 256], f32r, tag="wq")
        wk_sb = persist.tile([128, NKC, 256], f32r, tag="wk")
        wv_sb = persist.tile([128, NKC, 256], f32r, tag="wv")
        wp_sb = persist.tile([128, 2, C], f32r, tag="wp")
        qT = [persist.tile([128, T], f32r, tag=f"qT{p}", name=f"qT{p}") for p in range(2)]
        kT = [persist.tile([128, T], f32r, tag=f"kT{p}", name=f"kT{p}") for p in range(2)]
        vt = [persist.tile([128, NTS * 65], f32r, tag=f"vt{h}", name=f"vt{h}") for h in range(HPC)]
        attnT = [persist.tile([128, T], f32r, tag=f"attnT{p}", name=f"attnT{p}") for p in range(2)]

        # ---- loads: chunked, interleaved in consumption order, 2 HW queues ----
        nc.gpsimd.dma_start(out=wp_sb, in_=wp_d.rearrange("(k p) n -> p k n", p=128).bitcast(f32r))
        for eng, par in ((nc.sync, 0), (nc.scalar, 1)):
            for c in range(par, NKC, 2):
                eng.dma_start(out=wq_sb[:, c, :],
                              in_=wq_d[c * 128:(c + 1) * 128, :].bitcast(f32r))
            for c in range(par, NKC, 2):
                eng.dma_start(out=xt[c], in_=xT_d[c * 128:(c + 1) * 128, :].bitcast(f32r))
            for c in range(par, NKC, 2):
                eng.dma_start(out=wv_sb[:, c, :],
                              in_=wv_d[c * 128:(c + 1) * 128, :].bitcast(f32r))
            for c in range(par, NKC, 2):
                eng.dma_start(out=wk_sb[:, c, :],
                              in_=wk_d[c * 128:(c + 1) * 128, :].bitcast(f32r))

        ones16 = persist.tile([128, NTS, 1], f32, tag="ones16")
        nc.vector.memset(ones16, 1.0)
        # per-hh zero-padded q tiles: rows of the *other* head are zero so the
        # scores matmul can contract over all 128 partitions (full PE array)
        qTz = [persist.tile([128, 512], f32r, tag=f"qTz{hh}", name=f"qTz{hh}")
               for hh in range(2)]
        zeros128 = persist.tile([128, 512], f32, tag="zeros128")
        nc.vector.memset(zeros128, 0.0)
        nc.vector.tensor_copy(out=qTz[0][64:128, :], in_=zeros128[64:128, :])
        nc.vector.tensor_copy(out=qTz[1][0:64, :], in_=zeros128[0:64, :])

        # ---------- emission helpers ----------
        filler = []     # queue of closures emitting independent PE work

        def pull(n):
            for _ in range(n):
                if filler:
                    filler.pop(0)()

        def qk_chain_units(pair, dst, w_sb, J, name):
            # split one 8-matmul accumulation chain into 4 filler units
            ps = psaux.tile([128, 512], f32, tag="aux", name=name)

            def unit(c0):
                def f():
                    for c in (c0, c0 + 1):
                        nc.tensor.matmul(
                            ps,
                            lhsT=w_sb[:, c, 128 * pair:128 * pair + 128],
                            rhs=xt[c][:, 512 * J:512 * J + 512],
                            start=(c == 0), stop=(c == NKC - 1))
                    if c0 == NKC - 2:
                        nc.vector.tensor_copy(
                            out=dst[:, 512 * J:512 * J + 512], in_=ps)
                return f
            return [unit(c0) for c0 in range(0, NKC, 2)]

        def qk_chain(pair, dst, w_sb, J, name):
            for u in qk_chain_units(pair, dst, w_sb, J, name):
                u()

        def v_chain(t):
            ps = psaux.tile([128, 512], f32, tag="aux", name=f"v_{t}")
            for c in range(NKC):
                nc.tensor.matmul(
                    ps[:, 0:256],
                    lhsT=xt[c][:, 128 * t:128 * t + 128],
                    rhs=wv_sb[:, c, :],
                    start=(c == 0), stop=(c == NKC - 1))
            for h in range(HPC):
                nc.vector.tensor_copy(
                    out=vt[h][:, 65 * t:65 * t + 64], in_=ps[:, 64 * h:64 * h + 64])

        def proj_tile(m, n):
            py_ = psaux.tile([128, 512], f32, tag="aux", name=f"y_{m}_{n}")
            for pair in range(2):
                nc.tensor.matmul(
                    py_,
                    lhsT=attnT[pair][:, 128 * m:128 * m + 128],
                    rhs=wp_sb[:, pair, 512 * n:512 * n + 512],
                    start=(pair == 0), stop=(pair == 1))
            yo = outp.tile([128, 512], f32, tag="yo")
            nc.vector.tensor_copy(out=yo, in_=py_)
            nc.sync.dma_start(
                out=y_d[128 * m:128 * m + 128, 512 * n:512 * n + 512], in_=yo)

        def att_block(pair, hh, J):
            h = 2 * pair + hh
            nch = 4 * J + 4
            pa = psatt.tile([65, 512], f32, tag="att", name=f"pa_{h}_{J}")
            nc.vector.tensor_copy(
                out=qTz[hh][64 * hh:64 * hh + 64, :],
                in_=qT[pair][64 * hh:64 * hh + 64, 512 * J:512 * J + 512])
            pend = None          # (et, t0, t1) AV one step behind scores
            for u in range(nch // 2):
                t0, t1 = 2 * u, 2 * u + 1
                ss = psp.tile([128, 1024], f32, tag="s", name=f"ss_{h}_{J}_{u}")
                for half, t in ((0, t0), (1, t1)):
                    nc.tensor.matmul(
                        ss[:, 512 * half:512 * half + 512],
                        lhsT=kT[pair][:, 128 * t:128 * t + 128],
                        rhs=qTz[hh],
                        start=True, stop=True)
                et = work.tile([128, 1024], f32r, tag="et", bufs=3)
                nc.scalar.activation(out=et, in_=ss, func=EXP, scale=scale)
                for half, t in ((0, t0), (1, t1)):
                    if t >= 4 * J:
                        d = t - 4 * J
                        sl = et[:, 512 * half:512 * half + 512]
                        # keep el iff f >= p + 128*d:  (-1)*p + 1*f - 128*d >= 0
                        nc.gpsimd.affine_select(
                            out=sl, in_=sl,
                            compare_op=mybir.AluOpType.is_ge,
                            fill=0.0, base=-128 * d,
                            pattern=[[1, 512]], channel_multiplier=-1)
                if pend is not None:
                    for half, t in ((0, pend[1]), (1, pend[2])):
                        nc.tensor.matmul(
                            pa, lhsT=vt[h][:, 65 * t:65 * t + 65],
                            rhs=pend[0][:, 512 * half:512 * half + 512],
                            start=(t == 0), stop=False)
                pend = (et, t0, t1)
                pull(1)
            for half, t in ((0, pend[1]), (1, pend[2])):
                nc.tensor.matmul(
                    pa, lhsT=vt[h][:, 65 * t:65 * t + 65],
                    rhs=pend[0][:, 512 * half:512 * half + 512],
                    start=(t == 0), stop=(t == nch - 1))
            sums = small.tile([1, 512], f32, tag="sums")
            nc.scalar.copy(sums, pa[64:65, :])
            bsums = small.tile([64, 512], f32, tag="bsums")
            nc.gpsimd.partition_broadcast(bsums, sums)
            recip = small.tile([64, 512], f32, tag="recip")
            nc.vector.reciprocal_approx_fast(out=recip, in_=bsums)
            if hh == 0:
                nc.vector.tensor_mul(
                    attnT[pair][0:64, 512 * J:512 * J + 512], pa[0:64, :], recip)
            else:
                tmp = small.tile([64, 512], f32r, tag="tmp")
                nc.vector.tensor_mul(tmp, pa[0:64, :], recip)
                nc.sync.dma_start(
                    out=attnT[pair][64:128, 512 * J:512 * J + 512], in_=tmp)

        # ---------- phase A: qk(pair0) + v(all 4 heads) ----------
        for J in range(NJ):
            qk_chain(0, qT[0], wq_sb, J, f"q0_{J}")
        for t in range(NTS):
            v_chain(t)
        for J in range(NJ):
            qk_chain(0, kT[0], wk_sb, J, f"k0_{J}")
        for h in range(HPC):
            nc.vector.tensor_copy(
                out=vt[h].rearrange("p (t x) -> p t x", x=65)[:, :, 64:65],
                in_=ones16)

        # ---------- phase B: attention(pair0), gaps filled with qk(pair1) ----------
        for J in range(NJ):
            filler.extend(qk_chain_units(1, qT[1], wq_sb, J, f"q1_{J}"))
        for J in range(NJ):
            filler.extend(qk_chain_units(1, kT[1], wk_sb, J, f"k1_{J}"))
        for J in range(NJ):
            for hh in range(2):
                att_block(0, hh, J)
        pull(len(filler))

        # ---------- phase C: attention(pair1), gaps filled with proj ----------
        for J in range(NJ):
            att_block(1, 0, J)
            att_block(1, 1, J)
            filler.extend(
                (lambda m=m, n=n: (lambda: proj_tile(m, n)))()
                for m in range(4 * J, 4 * J + 4) for n in range(2))
        pull(len(filler))

    nc.compile()
    _NC_CACHE["nc"] = nc
    return nc


def _make_mask01():
    m = np.zeros((4, 128, 512), dtype=np.float32)
    p = np.arange(128)[:, None]
    f = np.arange(512)[None, :]
    for d in range(4):
        m[d] = (f >= 128 * d + p).astype(np.float32)
    return m


def make_in_maps(x, wq, wk, wv, wproj):
    xTs = [np.ascontiguousarray(x[b].T) for b in range(B)]
    in_maps = []
    for core in range(NCORES):
        b, g = divmod(core, 4)
        hs = slice(4 * g, 4 * g + 4)
        in_maps.append({
            "xT": xTs[b],
            "wq_s": np.ascontiguousarray(wq[hs].transpose(1, 0, 2).reshape(C, HPC * HS)),
            "wk_s": np.ascontiguousarray(wk[hs].transpose(1, 0, 2).reshape(C, HPC * HS)),
            "wv_s": np.ascontiguousarray(wv[hs].transpose(1, 0, 2).reshape(C, HPC * HS)),
            "wp_s": np.ascontiguousarray(wproj[4 * g * HS:(4 * g + 4) * HS, :]),
        })
    return in_maps


def _assemble(results, bproj):
    y = np.zeros((B, T, C), dtype=np.float32)
    for core in range(NCORES):
        y[core // 4] += results[core]["y"]
    y += bproj.astype(np.float32)[None, None, :]
    return y


def _is_causal(attention_mask):
    tril = np.tril(np.ones((T, T), dtype=bool))
    return all(np.array_equal(attention_mask[b], tril) for b in range(B))


def _numpy_fallback(x, attention_mask, wq, wk, wv, wproj, bproj):
    x64 = x.astype(np.float32)
    q = np.einsum('btc,hcd->bhtd', x64, wq)
    k = np.einsum('btc,hcd->bhtd', x64, wk)
    v = np.einsum('btc,hcd->bhtd', x64, wv)
    wei = np.einsum('bhtd,bhsd->bhts', q, k) / np.sqrt(np.float32(HS))
    wei = np.where(attention_mask[:, None, :, :], wei, -np.inf)
    wei = wei - wei.max(axis=-1, keepdims=True)
    wei = np.exp(wei)
    wei = wei / wei.sum(axis=-1, keepdims=True)
    out = np.einsum('bhts,bhsd->bhtd', wei, v)
    out = out.transpose(0, 2, 1, 3).reshape(B, T, H * HS)
    return (out @ wproj + bproj).astype(np.float32)


def _install_ntff_hook():
    """Recreate the antenv.axon_hooks shim so trace=True works under axon."""
    import sys, types
    try:
        from antenv.axon_hooks import get_axon_ntff_profile_hook  # noqa
        return
    except ImportError:
        pass
    import antenv
    mod = types.ModuleType("antenv.axon_hooks")
    holder = [None]
    mod.set_axon_ntff_profile_hook = lambda h: holder.__setitem__(0, h)
    mod.get_axon_ntff_profile_hook = lambda: holder[0]
    sys.modules["antenv.axon_hooks"] = mod
    antenv.axon_hooks = mod
    if "/root/.axon_site" not in sys.path:
        sys.path.insert(0, "/root/.axon_site")
    from trn_agent_boot.trn_boot import _ntff_profile_via_ctypes
    mod.set_axon_ntff_profile_hook(_ntff_profile_via_ctypes("/opt/axon/libaxon_pjrt.so"))


def kernel(x, attention_mask, wq, wk, wv, wproj, bproj, _trace=False):
    x = np.asarray(x); attention_mask = np.asarray(attention_mask)
    wq = np.asarray(wq); wk = np.asarray(wk); wv = np.asarray(wv)
    wproj = np.asarray(wproj); bproj = np.asarray(bproj)

    if not _is_causal(attention_mask):
        return _numpy_fallback(x, attention_mask, wq, wk, wv, wproj, bproj)

    from concourse import bass_utils
    if _trace:
        _install_ntff_hook()
        bass_utils.upload_artifacts = lambda d: d
    nc = _build_nc()
    in_maps = make_in_maps(x, wq, wk, wv, wproj)
    res = bass_utils.run_bass_kernel_spmd(
        nc, in_maps, core_ids=list(range(NCORES)), trace=_trace)
    out = _assemble(res.results, bproj)
    if _trace:
        return out, res
    return out


# revision 14
# speedup vs baseline: 1.5110x; 1.0760x over previous
"""Multi-head causal attention on 8 Trainium2 NeuronCores.

Problem: B=2, T=2048, C=1024, H=16, HS=64 (fp32), causal mask.

Sharding: 8 cores = 2 batches x 4 head-groups (4 heads each). Each core
computes q/k/v projections + attention + its partial output projection for
its 4 heads of its batch; the host sums the 4 per-batch partials (the
all-reduce of the tensor-parallel output projection) and adds the bias.

Per-core kernel dataflow (everything "transposed", T on the free axis):
  qT/kT [heads(64)x2, T] = W.T @ xT          (PE, K=C chunks of 128)
  v     [T, 64+ones]                         (PE)
  sT    [ts=128, tq=512] = kT.T-slice @ qT   (PE)  -> exp(s/8) (ACT)
  causal: multiplicative 0/1 mask tiles on the 4 diagonal ts-chunks (DVE)
  attnT_aug [65, tq] += v_aug.T @ expT       (PE, ones column => row 64 = softmax denom)
  recip = 1/denom (DVE), broadcast over 64 partitions via K=1 matmul (PE)
  attnT = attnT_aug[0:64] * recip            (DVE)  (odd head -> partition-shift DMA)
  y_partial [tq, C] = attnT_pair.T @ wproj   (PE, K=128 per head-pair)

float32r = full-precision fp32 matmul at 1 cycle/row (vs 4 for plain fp32).
"""

import numpy as np

B, T, C, H, HS = 2, 2048, 1024, 16, 64
NCORES = 8
HPC = 4            # heads per core
NKC = C // 128     # contraction chunks (8)
NJ = T // 512      # tq chunks (4)
NTS = T // 128     # ts chunks (16)

_NC_CACHE = {}


def _build_nc():
    if "nc" in _NC_CACHE:
        return _NC_CACHE["nc"]
    from contextlib import ExitStack
    import concourse.bass as bass
    from concourse import bacc, tile, mybir

    f32 = mybir.dt.float32
    f32r = mybir.dt.float32r
    EXP = mybir.ActivationFunctionType.Exp

    nc = bacc.Bacc("TRN2", target_bir_lowering=False, debug=False,
                   enable_asserts=False, num_devices=NCORES)

    xT_d = nc.dram_tensor("xT", (C, T), f32, kind="ExternalInput").ap()
    wq_d = nc.dram_tensor("wq_s", (C, HPC * HS), f32, kind="ExternalInput").ap()
    wk_d = nc.dram_tensor("wk_s", (C, HPC * HS), f32, kind="ExternalInput").ap()
    wv_d = nc.dram_tensor("wv_s", (C, HPC * HS), f32, kind="ExternalInput").ap()
    wp_d = nc.dram_tensor("wp_s", (HPC * HS, C), f32, kind="ExternalInput").ap()
    y_d = nc.dram_tensor("y", (T, C), f32, kind="ExternalOutput").ap()

    scale = float(1.0 / np.sqrt(HS))

    with tile.TileContext(nc) as tc, ExitStack() as ctx:
        persist = ctx.enter_context(tc.tile_pool(name="persist", bufs=1))
        work = ctx.enter_context(tc.tile_pool(name="work", bufs=3))
        small = ctx.enter_context(tc.tile_pool(name="small", bufs=2))
        outp = ctx.enter_context(tc.tile_pool(name="outp", bufs=2))
        psp = ctx.enter_context(tc.tile_pool(name="psp", bufs=2, space="PSUM"))
        psaux = ctx.enter_context(tc.tile_pool(name="psaux", bufs=2, space="PSUM"))
        psatt = ctx.enter_context(tc.tile_pool(name="psatt", bufs=2, space="PSUM"))

        # ---- persistent SBUF tensors (f32r = fast-fp32 PE path, ~1.6e-4) ----
        xt = [persist.tile([128, T], f32r, tag=f"xt{c}", name=f"xt{c}") for c in range(NKC)]
        wq_sb = persist.tile([128, NKC, 256], f32r, tag="wq")
        wk_sb = persist.tile([128, NKC, 256], f32r, tag="wk")
        wv_sb = persist.tile([128, NKC, 256], f32r, tag="wv")
        wp_sb = persist.tile([128, 2, C], f32r, tag="wp")
        qT = [persist.tile([128, T], f32r, tag=f"qT{p}", name=f"qT{p}") for p in range(2)]
        kT = [persist.tile([128, T], f32r, tag=f"kT{p}", name=f"kT{p}") for p in range(2)]
        vt = [persist.tile([128, NTS * 65], f32r, tag=f"vt{h}", name=f"vt{h}") for h in range(HPC)]
        attnT = [persist.tile([128, T], f32r, tag=f"attnT{p}", name=f"attnT{p}") for p in range(2)]

        # ---- loads: chunked, interleaved in consumption order, 2 HW queues ----
        nc.gpsimd.dma_start(out=wp_sb, in_=wp_d.rearrange("(k p) n -> p k n", p=128).bitcast(f32r))
        for eng, par in ((nc.sync, 0), (nc.scalar, 1)):
            for c in range(par, NKC, 2):
                eng.dma_start(out=wq_sb[:, c, :],
                              in_=wq_d[c * 128:(c + 1) * 128, :].bitcast(f32r))
            for c in range(par, NKC, 2):
                eng.dma_start(out=wv_sb[:, c, :],
                              in_=wv_d[c * 128:(c + 1) * 128, :].bitcast(f32r))
            for c in range(par, NKC, 2):
                eng.dma_start(out=xt[c][:, 0:1024],
                              in_=xT_d[c * 128:(c + 1) * 128, 0:1024].bitcast(f32r))
            for c in range(par, NKC, 2):
                eng.dma_start(out=wk_sb[:, c, :],
                              in_=wk_d[c * 128:(c + 1) * 128, :].bitcast(f32r))
            for c in range(par, NKC, 2):
                eng.dma_start(out=xt[c][:, 1024:2048],
                              in_=xT_d[c * 128:(c + 1) * 128, 1024:2048].bitcast(f32r))

        ones16 = persist.tile([128, NTS, 1], f32, tag="ones16")
        nc.vector.memset(ones16, 1.0)
        # per-hh zero-padded q tiles: rows of the *other* head are zero so the
        # scores matmul can contract over all 128 partitions (full PE array)
        qTz = [persist.tile([128, 512], f32r, tag=f"qTz{hh}", name=f"qTz{hh}")
               for hh in range(2)]
        zeros128 = persist.tile([128, 512], f32, tag="zeros128")
        nc.vector.memset(zeros128, 0.0)
        nc.vector.tensor_copy(out=qTz[0][64:128, :], in_=zeros128[64:128, :])
        nc.vector.tensor_copy(out=qTz[1][0:64, :], in_=zeros128[0:64, :])

        # ---------- emission helpers ----------
        filler = []     # queue of closures emitting independent PE work

        def pull(n):
            for _ in range(n):
                if filler:
                    filler.pop(0)()

        def qk_chain_units(pair, dst, w_sb, J, name):
            # split one 8-matmul accumulation chain into 4 filler units
            ps = psaux.tile([128, 512], f32, tag="aux", name=name)

            def unit(c0):
                def f():
                    for c in (c0, c0 + 1):
                        nc.tensor.matmul(
                            ps,
                            lhsT=w_sb[:, c, 128 * pair:128 * pair + 128],
                            rhs=xt[c][:, 512 * J:512 * J + 512],
                            start=(c == 0), stop=(c == NKC - 1))
                    if c0 == NKC - 2:
                        nc.vector.tensor_copy(
                            out=dst[:, 512 * J:512 * J + 512], in_=ps)
                return f
            return [unit(c0) for c0 in range(0, NKC, 2)]

        def qk_chain(pair, dst, w_sb, J, name):
            for u in qk_chain_units(pair, dst, w_sb, J, name):
                u()

        def v_chain(t):
            ps = psaux.tile([128, 512], f32, tag="aux", name=f"v_{t}")
            for c in range(NKC):
                nc.tensor.matmul(
                    ps[:, 0:256],
                    lhsT=xt[c][:, 128 * t:128 * t + 128],
                    rhs=wv_sb[:, c, :],
                    start=(c == 0), stop=(c == NKC - 1))
            for h in range(HPC):
                nc.vector.tensor_copy(
                    out=vt[h][:, 65 * t:65 * t + 64], in_=ps[:, 64 * h:64 * h + 64])

        def proj_tile(m, n):
            py_ = psaux.tile([128, 512], f32, tag="aux", name=f"y_{m}_{n}")
            for pair in range(2):
                nc.tensor.matmul(
                    py_,
                    lhsT=attnT[pair][:, 128 * m:128 * m + 128],
                    rhs=wp_sb[:, pair, 512 * n:512 * n + 512],
                    start=(pair == 0), stop=(pair == 1))
            yo = outp.tile([128, 512], f32, tag="yo")
            nc.vector.tensor_copy(out=yo, in_=py_)
            nc.sync.dma_start(
                out=y_d[128 * m:128 * m + 128, 512 * n:512 * n + 512], in_=yo)

        def att_block(pair, hh, J, extra=1):
            h = 2 * pair + hh
            nch = 4 * J + 4
            pa = psatt.tile([65, 512], f32, tag="att", name=f"pa_{h}_{J}")
            nc.vector.tensor_copy(
                out=qTz[hh][64 * hh:64 * hh + 64, :],
                in_=qT[pair][64 * hh:64 * hh + 64, 512 * J:512 * J + 512])
            pend = None          # (et, t0, t1) AV one step behind scores
            for u in range(nch // 2):
                t0, t1 = 2 * u, 2 * u + 1
                ss = psp.tile([128, 1024], f32, tag="s", name=f"ss_{h}_{J}_{u}")
                for half, t in ((0, t0), (1, t1)):
                    nc.tensor.matmul(
                        ss[:, 512 * half:512 * half + 512],
                        lhsT=kT[pair][:, 128 * t:128 * t + 128],
                        rhs=qTz[hh],
                        start=True, stop=True)
                et = work.tile([128, 1024], f32r, tag="et", bufs=3)
                nc.scalar.activation(out=et, in_=ss, func=EXP, scale=scale)
                for half, t in ((0, t0), (1, t1)):
                    if t >= 4 * J:
                        d = t - 4 * J
                        sl = et[:, 512 * half:512 * half + 512]
                        # keep el iff f >= p + 128*d:  (-1)*p + 1*f - 128*d >= 0
                        nc.gpsimd.affine_select(
                            out=sl, in_=sl,
                            compare_op=mybir.AluOpType.is_ge,
                            fill=0.0, base=-128 * d,
                            pattern=[[1, 512]], channel_multiplier=-1)
                if pend is not None:
                    for half, t in ((0, pend[1]), (1, pend[2])):
                        nc.tensor.matmul(
                            pa, lhsT=vt[h][:, 65 * t:65 * t + 65],
                            rhs=pend[0][:, 512 * half:512 * half + 512],
                            start=(t == 0), stop=False)
                pend = (et, t0, t1)
                pull(extra)
            for half, t in ((0, pend[1]), (1, pend[2])):
                nc.tensor.matmul(
                    pa, lhsT=vt[h][:, 65 * t:65 * t + 65],
                    rhs=pend[0][:, 512 * half:512 * half + 512],
                    start=(t == 0), stop=(t == nch - 1))
            sums = small.tile([1, 512], f32, tag="sums")
            nc.vector.tensor_copy(out=sums, in_=pa[64:65, :])
            bsums = small.tile([64, 512], f32, tag="bsums")
            nc.gpsimd.partition_broadcast(bsums, sums)
            recip = small.tile([64, 512], f32, tag="recip")
            nc.vector.reciprocal_approx_fast(out=recip, in_=bsums)
            if hh == 0:
                nc.vector.tensor_mul(
                    attnT[pair][0:64, 512 * J:512 * J + 512], pa[0:64, :], recip)
            else:
                tmp = small.tile([64, 512], f32r, tag="tmp")
                nc.vector.tensor_mul(tmp, pa[0:64, :], recip)
                nc.sync.dma_start(
                    out=attnT[pair][64:128, 512 * J:512 * J + 512], in_=tmp)

        # ---------- phase A: left-half (tq/ts < 1024) consumers ----------
        for h in range(HPC):
            nc.vector.tensor_copy(
                out=vt[h].rearrange("p (t x) -> p t x", x=65)[:, :, 64:65],
                in_=ones16)
        for J in (0, 1):
            qk_chain(0, qT[0], wq_sb, J, f"q0_{J}")
        for J in (0, 1):
            qk_chain(0, kT[0], wk_sb, J, f"k0_{J}")
        for t in range(8):
            v_chain(t)

        # ---------- phase B: attention(pair0); fillers = right-half qkv + qk(pair1) ----------
        for J in (2, 3):
            filler.extend(qk_chain_units(0, qT[0], wq_sb, J, f"q0_{J}"))
        for t in range(8, NTS):
            filler.append(lambda t=t: v_chain(t))
        for J in (2, 3):
            filler.extend(qk_chain_units(0, kT[0], wk_sb, J, f"k0_{J}"))
        for J in range(NJ):
            filler.extend(qk_chain_units(1, qT[1], wq_sb, J, f"q1_{J}"))
        for J in range(NJ):
            filler.extend(qk_chain_units(1, kT[1], wk_sb, J, f"k1_{J}"))
        # front-load: the J=0/1 blocks are small, so pull extra fillers there
        for J in range(NJ):
            for hh in range(2):
                att_block(0, hh, J, extra=2 if J < 2 else 1)
        pull(len(filler))

        # ---------- phase C: attention(pair1), gaps filled with proj ----------
        for J in range(NJ):
            att_block(1, 0, J)
            att_block(1, 1, J)
            filler.extend(
                (lambda m=m, n=n: (lambda: proj_tile(m, n)))()
                for m in range(4 * J, 4 * J + 4) for n in range(2))
        pull(len(filler))

    nc.compile()
    _NC_CACHE["nc"] = nc
    return nc


def _make_mask01():
    m = np.zeros((4, 128, 512), dtype=np.float32)
    p = np.arange(128)[:, None]
    f = np.arange(512)[None, :]
    for d in range(4):
        m[d] = (f >= 128 * d + p).astype(np.float32)
    return m


def make_in_maps(x, wq, wk, wv, wproj):
    xTs = [np.ascontiguousarray(x[b].T) for b in range(B)]
    in_maps = []
    for core in range(NCORES):
        b, g = divmod(core, 4)
        hs = slice(4 * g, 4 * g + 4)
        in_maps.append({
            "xT": xTs[b],
            "wq_s": np.ascontiguousarray(wq[hs].transpose(1, 0, 2).reshape(C, HPC * HS)),
            "wk_s": np.ascontiguousarray(wk[hs].transpose(1, 0, 2).reshape(C, HPC * HS)),
            "wv_s": np.ascontiguousarray(wv[hs].transpose(1, 0, 2).reshape(C, HPC * HS)),
            "wp_s": np.ascontiguousarray(wproj[4 * g * HS:(4 * g + 4) * HS, :]),
        })
    return in_maps


def _assemble(results, bproj):
    y = np.zeros((B, T, C), dtype=np.float32)
    for core in range(NCORES):
        y[core // 4] += results[core]["y"]
    y += bproj.astype(np.float32)[None, None, :]
    return y


def _is_causal(attention_mask):
    tril = np.tril(np.ones((T, T), dtype=bool))
    return all(np.array_equal(attention_mask[b], tril) for b in range(B))


def _numpy_fallback(x, attention_mask, wq, wk, wv, wproj, bproj):
    x64 = x.astype(np.float32)
    q = np.einsum('btc,hcd->bhtd', x64, wq)
    k = np.einsum('btc,hcd->bhtd', x64, wk)
    v = np.einsum('btc,hcd->bhtd', x64, wv)
    wei = np.einsum('bhtd,bhsd->bhts', q, k) / np.sqrt(np.float32(HS))
    wei = np.where(attention_mask[:, None, :, :], wei, -np.inf)
    wei = wei - wei.max(axis=-1, keepdims=True)
    wei = np.exp(wei)
    wei = wei / wei.sum(axis=-1, keepdims=True)
    out = np.einsum('bhts,bhsd->bhtd', wei, v)
    out = out.transpose(0, 2, 1, 3).reshape(B, T, H * HS)
    return (out @ wproj + bproj).astype(np.float32)


def _install_ntff_hook():
    """Recreate the antenv.axon_hooks shim so trace=True works under axon."""
    import sys, types
    try:
        from antenv.axon_hooks import get_axon_ntff_profile_hook  # noqa
        return
    except ImportError:
        pass
    import antenv
    mod = types.ModuleType("antenv.axon_hooks")
    holder = [None]
    mod.set_axon_ntff_profile_hook = lambda h: holder.__setitem__(0, h)
    mod.get_axon_ntff_profile_hook = lambda: holder[0]
    sys.modules["antenv.axon_hooks"] = mod
    antenv.axon_hooks = mod
    if "/root/.axon_site" not in sys.path:
        sys.path.insert(0, "/root/.axon_site")
    from trn_agent_boot.trn_boot import _ntff_profile_via_ctypes
    mod.set_axon_ntff_profile_hook(_ntff_profile_via_ctypes("/opt/axon/libaxon_pjrt.so"))


def kernel(x, attention_mask, wq, wk, wv, wproj, bproj, _trace=False):
    x = np.asarray(x); attention_mask = np.asarray(attention_mask)
    wq = np.asarray(wq); wk = np.asarray(wk); wv = np.asarray(wv)
    wproj = np.asarray(wproj); bproj = np.asarray(bproj)

    if not _is_causal(attention_mask):
        return _numpy_fallback(x, attention_mask, wq, wk, wv, wproj, bproj)

    from concourse import bass_utils
    if _trace:
        _install_ntff_hook()
        bass_utils.upload_artifacts = lambda d: d
    nc = _build_nc()
    in_maps = make_in_maps(x, wq, wk, wv, wproj)
    res = bass_utils.run_bass_kernel_spmd(
        nc, in_maps, core_ids=list(range(NCORES)), trace=_trace)
    out = _assemble(res.results, bproj)
    if _trace:
        return out, res
    return out
